# revision 1
# baseline (speedup 1.0000x reference)
"""CosAttention TRN2 kernel: qkv projection + cosine-sim attention.

Sharding: pure data-parallel over batch (B=8 -> one batch element per core).
Each core computes its full [N, C] output locally; no collectives.

Per-core layout strategy:
  - host passes xT = x[b].T [C, N] and WT = W_qkv.T [C, 3C]
  - qT/kT computed transposed (head dim on partitions) so the scores matmul
    S.T[keys, queries] = kn.T @ qn needs no on-device transposes
  - cosine scores lie in [-1, 1] so softmax needs no max subtraction
  - v computed in [tokens, C] layout with an appended ones column per head:
    the PV matmul then yields the softmax denominator for free
  - kernel writes outT [C, N]; host transposes back
"""

import sys

sys.path.insert(0, "/opt/trn_rl_repo")

from contextlib import ExitStack

import numpy as np

import concourse.bass as bass
import concourse.tile as tile
from concourse import bacc, mybir
from concourse.bass_utils import run_bass_kernel_spmd

B, N, C = 8, 1568, 768
H, D = 12, 64
NC3 = 3 * C  # 2304
QK = 2 * C  # 1536
F32 = mybir.dt.float32
F32R = mybir.dt.float32r
BF16 = mybir.dt.bfloat16

import os as _os

def _knob(name, dflt):
    return int(_os.environ.get(name, dflt))

NT = 13  # token tiles of 128 (last is 32 wide)
KC = 6  # contraction chunks of 128 over C
QBLKS = [(0, 512), (512, 512), (1024, 512), (1536, 32)]

ADD = mybir.AluOpType.add
MULT = mybir.AluOpType.mult
AF = mybir.ActivationFunctionType


def _tw(t):
    return 32 if t == NT - 1 else 128


def _r(ap):
    # view fp32 data as float32r so PE runs at 1 cycle/row
    return ap.bitcast(F32R)


def _build():
    # Bacc (not raw Bass): its compile() pass legalizes multi-semaphore waits
    # (event semaphores, matmul-wait -> ldweights moves) that walrus requires
    nc = bacc.Bacc()
    xT = nc.dram_tensor("xT", [C, N], F32, kind="ExternalInput")
    WT = nc.dram_tensor("WT", [C, NC3], F32, kind="ExternalInput")
    bqkv = nc.dram_tensor("bqkv", [NC3], F32, kind="ExternalInput")
    econst = nc.dram_tensor("econst", [2, 128], F32, kind="ExternalInput")
    out = nc.dram_tensor("out", [N, C], F32, kind="ExternalOutput")

    with ExitStack() as ctx:
        tc = ctx.enter_context(tile.TileContext(nc))
        persist = ctx.enter_context(tc.tile_pool(name="persist", bufs=1))

        # persistent SBUF tensors
        qn = persist.tile([128, 6, N], BF16)  # normalized q, 2 heads per tile row
        kn = persist.tile([128, 6, N], BF16)
        vaug = persist.tile([128, NT, H, D + 1], BF16)  # v + ones column per head
        bqk_t = persist.tile([128, 12], F32)  # q/k bias, per M-tile column
        bv_b = persist.tile([128, C], F32)  # v bias broadcast to all partitions
        # emat[k, p]: broadcast matmul selector — row k of the rhs lands on
        # partition half k of the output (host-provided; device ops cannot
        # write APs starting at partition 1)
        emat = persist.tile([2, 128], F32R)
        emat_f = persist.tile([2, 128], F32)
        gmat = persist.tile([128, 2], F32R)  # per-head sumsq reducer
        gmat_f = persist.tile([128, 2], F32)

        nc.sync.dma_start(out=emat_f, in_=econst[:, :])
        nc.vector.memset(gmat_f, 0.0)
        nc.vector.memset(gmat_f[0:64, 0:1], 1.0)
        nc.vector.memset(gmat_f[64:128, 1:2], 1.0)
        # memset/DMA cannot produce float32r; DVE copy-cast can
        with nc.allow_low_precision("f32r is bit-identical to f32"):
            nc.vector.tensor_copy(emat, emat_f)
            nc.vector.tensor_copy(gmat, gmat_f)
        # set everything to 1.0 (contiguous memset); the v eviction below
        # overwrites the v columns, leaving 1.0 only in each head's ones column
        nc.vector.memset(vaug, 1.0)

        nc.sync.dma_start(out=bqk_t, in_=bqkv[0:QK].rearrange("(t p) -> p t", p=128))
        bv_src = bqkv[QK:NC3]
        nc.sync.dma_start(
            out=bv_b,
            in_=bass.AP(tensor=bv_src.tensor, offset=bv_src.offset, ap=[[0, 128]] + list(bv_src.ap)),
        )

        # ---------------- phase 1: projections + normalization ----------------
        with (
            tc.tile_pool(name="ph1", bufs=1) as p1,
            tc.tile_pool(name="wtq", bufs=_knob("WTQ_BUFS", 2)) as wtq,
            tc.tile_pool(name="qkt", bufs=_knob("QKT_BUFS", 2)) as qktp,
            tc.tile_pool(name="sqp", bufs=_knob("SQ_BUFS", 8)) as sqp,
            tc.tile_pool(name="rr", bufs=2) as rrp,
        ):
            # stage DMA loads, then DVE-copy into the f32r tiles consumed by
            # PE: a self-loading f32r matmul has a single sync-wait slot, so
            # all its producers must collapse onto one (DVE) semaphore
            xs = p1.tile([128, KC, N], F32R)
            with tc.tile_pool(name="stage", bufs=_knob("STG_BUFS", 2)) as stg:
                for c in range(KC):
                    st = stg.tile([128, N], F32, tag="xstage")
                    nc.sync.dma_start(out=st, in_=xT[c * 128 : (c + 1) * 128, :])
                    with nc.allow_low_precision("f32r is bit-identical to f32"):
                        nc.vector.tensor_copy(xs[:, c, :], st)

            # v projection: out[tokens, 768] = xT.T @ WT[:, v]
                wv_all = p1.tile([128, KC, C], F32R)
                for c in range(KC):
                    st = stg.tile([128, C], F32, tag="wvstage")
                    nc.sync.dma_start(out=st, in_=WT[c * 128 : (c + 1) * 128, QK:NC3])
                    with nc.allow_low_precision("f32r is bit-identical to f32"):
                        nc.vector.tensor_copy(wv_all[:, c, :], st)
            with tc.tile_pool(name="pjv", bufs=_knob("PJV_BUFS", 2), space="PSUM") as pjvp:
                for t in range(NT):
                    w = _tw(t)
                    ps = pjvp.tile([128, 768], F32)
                    for c in range(KC):
                        lhs = xs[:, c, t * 128 : t * 128 + w]
                        nc.tensor.matmul(
                            ps[0:w, 0:512], lhs, wv_all[:, c, 0:512],
                            start=(c == 0), stop=(c == KC - 1),
                        )
                        nc.tensor.matmul(
                            ps[0:w, 512:768], lhs, wv_all[:, c, 512:768],
                            start=(c == 0), stop=(c == KC - 1),
                        )
                    nc.vector.tensor_add(
                        vaug[0:w, t, :, 0:D],
                        ps[0:w, :].rearrange("p (h d) -> p h d", d=D),
                        bv_b[0:w, :].rearrange("p (h d) -> p h d", d=D),
                    )

            # q/k projection per 128-row tile of W (2 heads each), then normalize
            qk_psums = ExitStack()
            pjp = qk_psums.enter_context(tc.tile_pool(name="pj", bufs=_knob("PJ_BUFS", 2), space="PSUM"))
            ssqp = qk_psums.enter_context(tc.tile_pool(name="ssq", bufs=2, space="PSUM"))
            bcp = qk_psums.enter_context(tc.tile_pool(name="bc", bufs=2, space="PSUM"))
            # two-stage software pipeline over the 12 W-row tiles: stage A
            # projects tile m and squares it; stage B (emitted between m+1's
            # matmuls) reduces/normalizes tile m so PE never stalls on the
            # DVE/ACT chain of the same tile
            state = {}

            def qk_stage_a(m):
                qk_t = qktp.tile([128, N], F32, tag="qk_t", name=f"qk_t{m}")
                wq = wtq.tile([128, KC, 128], F32R, tag="wq", name=f"wq{m}")
                wqs = wtq.tile([128, KC, 128], F32, tag="wqstage", name=f"wqs{m}")
                nc.sync.dma_start(
                    out=wqs,
                    in_=WT[:, m * 128 : (m + 1) * 128].rearrange("(c p) n -> p c n", p=128),
                )
                with nc.allow_low_precision("f32r is bit-identical to f32"):
                    nc.vector.tensor_copy(wq, wqs)
                sqs = []
                for (q0, qw) in QBLKS:
                    ps = pjp.tile([128, 512], F32, tag="pj512", name=f"pj{m}_{q0}")
                    for c in range(KC):
                        nc.tensor.matmul(
                            ps[:, 0:qw], wq[:, c, :], xs[:, c, q0 : q0 + qw],
                            start=(c == 0), stop=(c == KC - 1),
                        )
                    # add bias while evicting
                    nc.vector.tensor_scalar(
                        out=qk_t[:, q0 : q0 + qw], in0=ps[:, 0:qw],
                        scalar1=bqk_t[:, m : m + 1], scalar2=None, op0=ADD,
                    )
                    sq = sqp.tile([128, 512], F32R, tag="sq", name=f"sq{m}_{q0}")
                    nc.vector.tensor_mul(
                        sq[:, 0:qw], qk_t[:, q0 : q0 + qw], qk_t[:, q0 : q0 + qw]
                    )
                    sqs.append(sq)
                state[m] = (qk_t, sqs)

            def qk_stage_b(m):
                qk_t, sqs = state.pop(m)
                nrm = rrp.tile([2, N], F32, tag="nrm", name=f"nrm{m}")
                for i, (q0, qw) in enumerate(QBLKS):
                    sps = ssqp.tile([2, 512], F32, tag="sps", name=f"sps{m}_{q0}")
                    nc.tensor.matmul(
                        sps[:, 0:qw], gmat, sqs[i][:, 0:qw], start=True, stop=True
                    )
                    nc.scalar.activation(
                        out=nrm[:, q0 : q0 + qw], in_=sps[:, 0:qw], func=AF.Sqrt
                    )
                rinv = rrp.tile([2, N], F32R, tag="rinv", name=f"rinv{m}")
                with nc.allow_low_precision("f32r is bit-identical to f32"):
                    nc.vector.reciprocal(out=rinv, in_=nrm)
                dst = qn[:, m, :] if m < 6 else kn[:, m - 6, :]
                for (q0, qw) in QBLKS:
                    bc = bcp.tile([128, 512], F32, tag="bc", name=f"bc{m}_{q0}")
                    nc.tensor.matmul(
                        bc[:, 0:qw], emat, rinv[:, q0 : q0 + qw],
                        start=True, stop=True,
                    )
                    nc.vector.tensor_mul(
                        dst[:, q0 : q0 + qw], qk_t[:, q0 : q0 + qw], bc[:, 0:qw]
                    )

            for m in range(12):
                qk_stage_a(m)
                if m > 0:
                    qk_stage_b(m - 1)
            qk_stage_b(11)
            qk_psums.close()

        # ---------------- phase 2: attention ----------------
        with (
            tc.tile_pool(name="pt", bufs=_knob("PT_BUFS", 2)) as ptp,
            tc.tile_pool(name="ostg", bufs=4) as ostg,
            tc.tile_pool(name="lrow", bufs=4) as lrp,
            tc.tile_pool(name="sc", bufs=_knob("SC_BUFS", 3), space="PSUM") as scp,
            tc.tile_pool(name="pv", bufs=_knob("PV_BUFS", 2), space="PSUM") as pvp,
        ):
            def emit_scores(h, pt, t):
                # S.T[keys, queries] for one key tile, then exp -> bf16 P.T.
                # Two half-width psum tiles (2 banks each, double-buffered in
                # the pool) keep ACT exp ops back-to-back instead of
                # ping-ponging with the next k-tile's score matmuls.
                hp = (h % 2) * 64
                qh = qn[hp : hp + 64, h // 2, :]
                kh = kn[hp : hp + 64, h // 2, :]
                w = _tw(t)
                for half in range(2):
                    base = half * 784
                    sc = scp.tile([128, 784], F32, tag="sc")
                    for (b0, bw) in ((0, 512), (512, 272)):
                        nc.tensor.matmul(
                            sc[0:w, b0 : b0 + bw],
                            kh[:, t * 128 : t * 128 + w],
                            qh[:, base + b0 : base + b0 + bw],
                            start=True, stop=True,
                        )
                    # exp(cos sims): bounded inputs, no max subtraction needed
                    nc.scalar.activation(
                        out=pt[0:w, t, base : base + 784], in_=sc[0:w, :], func=AF.Exp
                    )

            def emit_pv(h, pt, qt):
                q0 = qt * 128
                qw = _tw(qt)
                po = pvp.tile([128, 65], F32, tag="po")
                for t in range(NT):
                    w = _tw(t)
                    nc.tensor.matmul(
                        po[0:qw, :],
                        pt[0:w, t, q0 : q0 + qw],
                        vaug[0:w, t, h, :],
                        start=(t == 0), stop=(t == NT - 1),
                    )
                linv = lrp.tile([128, 1], F32)
                nc.vector.reciprocal(out=linv[0:qw, :], in_=po[0:qw, 64:65])
                ot = ostg.tile([128, 64], F32)
                nc.vector.tensor_scalar(
                    out=ot[0:qw, :], in0=po[0:qw, 0:64],
                    scalar1=linv[0:qw, :], scalar2=None, op0=MULT,
                )
                nc.sync.dma_start(
                    out=out[q0 : q0 + qw, h * 64 : (h + 1) * 64], in_=ot[0:qw, :]
                )

            # software pipeline: PE instruction order interleaves head h's
            # score matmuls (which pace the ACT exp stream) with head h-1's
            # PV accumulation, so PE fills ACT-bound gaps instead of stalling
            pts = {}
            for h in range(H + 1):
                if h < H:
                    pts[h] = ptp.tile([128, NT, N], BF16, tag="pt", name=f"pt{h}")
                for t in range(NT if h < H else 0):
                    emit_scores(h, pts[h], t)
                    if h > 0:
                        emit_pv(h - 1, pts[h - 1], t)
                if h > 0:
                    if h == H:
                        for qt in range(NT):
                            emit_pv(h - 1, pts[h - 1], qt)
                    del pts[h - 1]
    nc.compile()
    return nc


_PROGRAM = None


def _get_program():
    global _PROGRAM
    if _PROGRAM is None:
        _PROGRAM = _build()
    return _PROGRAM


_LAST_RESULTS = None


def kernel(x, W_qkv, b_qkv):
    global _LAST_RESULTS
    nc = _get_program()
    xT = np.ascontiguousarray(np.transpose(np.asarray(x, np.float32), (0, 2, 1)))
    WTh = np.ascontiguousarray(np.asarray(W_qkv, np.float32).T)
    bh = np.ascontiguousarray(np.asarray(b_qkv, np.float32))
    ec = np.zeros((2, 128), np.float32)
    ec[0, 0:64] = 1.0
    ec[1, 64:128] = 1.0
    in_maps = [{"xT": xT[b], "WT": WTh, "bqkv": bh, "econst": ec} for b in range(B)]
    res = run_bass_kernel_spmd(nc, in_maps, core_ids=list(range(B)))
    _LAST_RESULTS = res
    o = np.stack([np.asarray(res.results[b]["out"]) for b in range(B)], axis=0)
    return np.ascontiguousarray(o.astype(np.float32))


if __name__ == "__main__":
    _build()
    print("build OK")

